# revision 47
# baseline (speedup 1.0000x reference)
"""DSPA (dual-softmax sparse attention) Trainium2 kernel.

Math (reference):
  x1 = x.reshape(2, 64, 4096)                       # [b, c, n]
  x2 = dilated_conv1d(x1, w, b) (k=3, dil=2, pad=1) # [b, c, m], m = n-2
  s[b] = x1[b]^T @ x2[b]                            # [n, m]
  x3 = softmax over b (2 elements)  ->  x3[0] = sigmoid(s0 - s1), x3[1] = 1 - x3[0]
  x4[b] = x2[b] @ x3[b]^T                           # [c, n]
  out = x1 + x4

Key identities used:
  p := sigmoid(d),  d := s0 - s1
  x4[0] = x2[0] @ p^T
  x4[1] = S1 - x2[1] @ p^T          with S1[c] = sum_j x2[1][c, j]
and in fp8 mode, with q := p - 0.5 = 0.5*tanh(d/2):
  x4[0] = x2[0] @ q^T + 0.5*S0
  x4[1] = 0.5*S1 - x2[1] @ q^T

Device strategy (8 cores, no collectives): shard the query dim n=4096 ->
512 columns per core. Each core computes the full conv x2 on-chip in a
batch-stacked, batch1-NEGATED bf16 layout x2bf [128, 4096] so one matmul
with K=128 yields d^T tiles directly.

Scheduling notes (from HW traces):
 - Every DMA piece lands ~2us after its (serialized, ~0.7us) issue, so
   the first misc piece carries conv weights AND a copy of xin[:, 0:520]
   -- everything conv(0) needs -- and later pieces are ordered by first
   use.
 - The PE p-state resets on even ~0.4us idle gaps (costing us of
   re-ramp), so garbage-transpose warmup runs until the first piece
   lands, and conv runs TWO chunks ahead of the consuming pipeline so no
   PE instruction ever waits on the conv-evacuation DVE chain.
 - Stage-3 lags the d-matmuls by 3 groups so PE never waits on the ACT
   sigmoid.
"""

import numpy as np
from collections import deque

import concourse.bacc as bacc
import concourse.mybir as mybir
import concourse.tile as tile
from concourse.bass_utils import run_bass_kernel_spmd

F32 = mybir.dt.float32
BF16 = mybir.dt.bfloat16
F8E4 = mybir.dt.float8e4

B, C, N, M = 2, 64, 4096, 4094
NCORES = 8
ISL = N // NCORES          # 512 query columns per core
NT = 32                    # j tiles of 128 (last has 2 zero cols)
NCH = 8                    # conv chunks of 512 (last = 510)
# misc layout: [w0 w1 w2 (384) | bias (2) | xin0copy (520) | ident (128)
#               | x1q (512)]
XIN0 = 386                 # xin[:, 0:520] copy offset
IDENT = 906
X1Q = 1034
MISC_W = X1Q + ISL         # 1546
NWARM = 26                 # PE p-state warmup transposes
FP8_S3 = True              # stage-3 matmuls in fp8e4 DoubleRow (K=256)

ADD = mybir.AluOpType.add
MULT = mybir.AluOpType.mult


def build_nc():
    nc = bacc.Bacc()

    xin = nc.dram_tensor("xin", [128, N + 2], BF16, kind="ExternalInput")
    misc = nc.dram_tensor("misc", [128, MISC_W], BF16, kind="ExternalInput")
    out = nc.dram_tensor("out", [128, ISL], BF16, kind="ExternalOutput")

    with tile.TileContext(nc) as tc:
        with (
            tc.tile_pool(name="persist", bufs=1) as persist,
            tc.tile_pool(name="ptiles", bufs=16) as ptiles,
            tc.tile_pool(name="psum_acc", bufs=1, space="PSUM") as psum_acc,
        ):
            x1pads = persist.tile([128, N + 2], BF16)
            msb = persist.tile([128, MISC_W], BF16)
            x2bf = persist.tile([128, NT * 128], BF16)
            x2T = persist.tile([128, NT * 128], F8E4 if FP8_S3 else BF16)
            wsrc = persist.tile([128, 128], BF16)
            scratch = persist.tile([128, 1], F32)
            s1parts = persist.tile([128, NCH + 1], F32)
            s1corr = persist.tile([128, 1], F32)
            s1half = persist.tile([128, 1], F32)
            osb = persist.tile([128, ISL], BF16)

            def wsb(k):
                return msb[:, 128 * k : 128 * (k + 1)]

            bsb = msb[:, 384:386].bitcast(F32)
            isb = msb[:, IDENT : IDENT + 128]
            x1q = msb[:, X1Q : X1Q + ISL]

            # Early memsets on the (otherwise idle) GPSIMD engine.
            nc.gpsimd.memset(wsrc[:, :], 0.0)
            nc.gpsimd.memset(scratch[:, :], 0.0)
            nc.gpsimd.memset(x2bf[:, M : NT * 128], 0.0)
            nc.gpsimd.memset(s1corr[:, :], 0.0)

            # Dummy activation: forces the ACT table DMA at kernel start.
            nc.scalar.activation(
                scratch[:, :], scratch[:, :],
                mybir.ActivationFunctionType.Tanh if FP8_S3
                else mybir.ActivationFunctionType.Sigmoid,
            )

            # Input DMAs ordered by first use. Piece 1 alone unblocks
            # conv(0); piece 2 (ident + x1q) unblocks d(0)/tr(0); piece 3
            # unblocks conv(1).
            nc.sync.dma_start(msb[:, 0:IDENT], misc[:, 0:IDENT])
            nc.sync.dma_start(msb[:, IDENT:], misc[:, IDENT:])
            nc.sync.dma_start(x1pads[:, 512:1028], xin[:, 512:1028])
            nc.sync.dma_start(x1pads[:, 1028:1720], xin[:, 1028:1720])
            nc.sync.dma_start(x1pads[:, 1720:2920], xin[:, 1720:2920])
            nc.sync.dma_start(x1pads[:, 2920:], xin[:, 2920:])

            # PE p-state warmup: garbage 128-row transposes keep the PE busy
            # (and ramping to 2.4GHz) until the first input piece lands.
            with tc.tile_pool(name="psum_warm", bufs=1, space="PSUM") as psum_warm:
                wps = psum_warm.tile([128, 128], BF16)
                for _ in range(NWARM):
                    nc.tensor.transpose(wps[:, :], wsrc[:, :], wsrc[:, :])

            acc = psum_acc.tile([128, ISL], F32)
            with (
                tc.tile_pool(name="psum_conv", bufs=2, space="PSUM") as psum_conv,
                tc.tile_pool(name="psum_tr", bufs=1, space="PSUM") as psum_tr,
                tc.tile_pool(name="psum_d", bufs=2, space="PSUM") as psum_d,
            ):
                pmap = {}
                s3q = deque()
                state = {"first": True}

                def emit_s3(g, last=False):
                    p = pmap.pop(g)
                    if FP8_S3:
                        lhs = x2T[:, 256 * g : 256 * (g + 1)].rearrange(
                            "p (t n) -> p t n", t=2
                        )
                        rhs = p[:, :].rearrange("p (t n) -> p t n", t=2)
                        nc.tensor.matmul(
                            acc[:, :], lhs, rhs,
                            start=state["first"], stop=last,
                            perf_mode=mybir.MatmulPerfMode.DoubleRow,
                        )
                        state["first"] = False
                    else:
                        for u in (0, 1):
                            t = 2 * g + u
                            nc.tensor.matmul(
                                acc[:, :],
                                x2T[:, 128 * t : 128 * (t + 1)],
                                p[:, 512 * u : 512 * u + 512],
                                start=state["first"], stop=(last and u == 1),
                            )
                            state["first"] = False

                def emit_conv(ch):
                    # Conv: x2bf[:, j] = sum_k wblkT[k].T @ x1pads[:, j + 2k]
                    # (block-diagonal weights, batch-1 block negated), plus
                    # its DVE evacuation (+bias, per-chunk row sums). Chunk
                    # 0 reads the xin copy embedded in misc piece 1, and its
                    # evacuation is split in halves so the first d-pair (and
                    # hence the first sigmoid) starts half an evac earlier.
                    j0 = ch * 512
                    w = min(512, M - j0)
                    pc = psum_conv.tile([128, 512], F32, name=f"pc{ch}", tag="pc")
                    for k in range(3):
                        if ch == 0:
                            rhs = msb[:, XIN0 + 2 * k : XIN0 + 2 * k + w]
                        else:
                            rhs = x1pads[:, j0 + 2 * k : j0 + 2 * k + w]
                        nc.tensor.matmul(
                            pc[:, 0:w], wsb(k), rhs,
                            start=(k == 0), stop=(k == 2),
                        )
                    halves = [(0, 256, 0), (256, 512, 1)] if ch == 0 else [(0, w, ch + 1)]
                    for a, b, col in halves:
                        nc.vector.tensor_scalar(
                            x2bf[:, j0 + a : j0 + b],
                            pc[:, a:b],
                            bsb,
                            0.0,
                            op0=ADD,
                            op1=ADD,
                            accum_out=s1parts[:, col : col + 1],
                        )

                # Conv runs ~2 chunks ahead of the consuming pipeline so the
                # transposes/d-matmuls never wait on the conv evacuation
                # chain, and the PE never idles waiting for ACT/DVE.
                # conv(1)/conv(2) go at iteration 0's tail (their DMA pieces
                # land after d(0)'s inputs; d(0) must not queue behind them).
                emit_conv(0)

                def emit_d(gg):
                    # d^T tiles + sigmoid/tanh. High priority: the sigmoid
                    # chain is the saturated resource, so the scheduler must
                    # prefer d-matmuls over transposes/conv when both are
                    # ready.
                    with tc.high_priority(offset=30):
                        dps = psum_d.tile([128, 1024], F32, name=f"d{gg}", tag="d")
                        for u in (0, 1):
                            t = 2 * gg + u
                            nc.tensor.matmul(
                                dps[:, 512 * u : 512 * u + 512],
                                x2bf[:, 128 * t : 128 * t + 128],
                                x1q,
                                start=True,
                                stop=True,
                            )
                        p = ptiles.tile(
                            [128, 1024], F8E4 if FP8_S3 else BF16,
                            name=f"p{gg}", tag="p",
                        )
                        if FP8_S3:
                            nc.scalar.activation(
                                p[:, :], dps[:, :],
                                mybir.ActivationFunctionType.Tanh, scale=0.5,
                            )
                        else:
                            nc.scalar.activation(
                                p[:, :], dps[:, :],
                                mybir.ActivationFunctionType.Sigmoid,
                            )
                        pmap[gg] = p
                        s3q.append(gg)

                for ch in range(NCH):
                    # Both d groups at the absolute iteration head: their
                    # sigmoids are the critical resource (ACT is saturated),
                    # and the d-pair must complete before ACT finishes the
                    # previous group's sigmoid.
                    emit_d(2 * ch)
                    emit_d(2 * ch + 1)
                    # Lagged stage-3: sigmoid of these groups completed at
                    # least one chunk ago.
                    while s3q and s3q[0] <= 2 * ch - 3:
                        emit_s3(s3q.popleft())
                    # x2^T tiles: 4 PE transposes into one PSUM tile, one
                    # DVE evacuation.
                    trt = psum_tr.tile([128, 1024], BF16, name=f"tr{ch}", tag="tr")
                    for tt in range(4):
                        t = 4 * ch + tt
                        nc.tensor.transpose(
                            trt[:, 128 * tt : 128 * (tt + 1)],
                            x2bf[:, 128 * t : 128 * (t + 1)],
                            isb,
                        )
                    # High priority: the fp8 x2T tiles feed stage-3; they
                    # must jump ahead of queued conv evacuations on DVE.
                    with tc.high_priority(offset=20):
                        if FP8_S3:
                            # x2T tiles scaled by 0.5 (folds q = 0.5*tanh(d/2)).
                            nc.vector.tensor_scalar(
                                x2T[:, 512 * ch : 512 * ch + 512], trt[:, 0:512],
                                0.5, 0.0, op0=MULT, op1=ADD,
                            )
                        else:
                            nc.vector.tensor_copy(
                                x2T[:, 512 * ch : 512 * ch + 512], trt[:, 0:512]
                            )
                    # All conv chunks front-loaded into iterations 0-2 (each
                    # gated only by its DMA piece landing): iterations 3-7
                    # are conv-free, so PE per iter drops below ACT's
                    # ~2.05us and the sigmoid chain runs bubble-free.
                    if ch == 0:
                        emit_conv(1)
                        emit_conv(2)
                    elif ch == 1:
                        emit_conv(3)
                        emit_conv(4)
                        emit_conv(5)
                    elif ch == 2:
                        emit_conv(6)
                    elif ch == 3:
                        emit_conv(7)
                    if ch == NCH - 1:
                        # Corrections, ready well before the epilogue.
                        if FP8_S3:
                            # corr = [0.5*S0 ; 0.5*S1]; s1parts rows hold
                            # [S0_chunk ; -S1_chunk].
                            nc.vector.reduce_sum(
                                s1corr[0:64, 0:1], s1parts[0:64, :],
                                axis=mybir.AxisListType.X,
                            )
                            nc.vector.reduce_sum(
                                s1corr[64:128, 0:1], s1parts[64:128, :],
                                axis=mybir.AxisListType.X, negate=True,
                            )
                            nc.vector.tensor_scalar(
                                s1half[:, :], s1corr[:, :], 0.5, 0.0,
                                op0=MULT, op1=ADD,
                            )
                        else:
                            # corr = [0 ; S1] (s1corr top was memset to 0).
                            nc.vector.reduce_sum(
                                s1corr[64:128, 0:1], s1parts[64:128, :],
                                axis=mybir.AxisListType.X, negate=True,
                            )
                while s3q:
                    g = s3q.popleft()
                    emit_s3(g, last=(len(s3q) == 0))
                # Epilogue: out = (acc + corr) + x1, in two halves so the
                # first half's DMA issue overlaps the second half's DVE pass.
                corr = s1half if FP8_S3 else s1corr
                for a, b in ((0, 256), (256, ISL)):
                    nc.vector.scalar_tensor_tensor(
                        osb[:, a:b],
                        acc[:, a:b],
                        corr[:, 0:1],
                        x1q[:, a:b],
                        op0=ADD,
                        op1=ADD,
                    )
                    nc.sync.dma_start(out[:, a:b], osb[:, a:b])

    nc.finalize()
    return nc


_NC_CACHE = None


def _get_nc():
    global _NC_CACHE
    if _NC_CACHE is None:
        _NC_CACHE = build_nc()
    return _NC_CACHE


def _host_prep(x, conv_w, conv_b):
    import ml_dtypes

    x1 = np.zeros((B * C, N + 2), dtype=np.float32)
    x1[:, 1 : N + 1] = x.reshape(B * C, N)
    x1 = x1.astype(ml_dtypes.bfloat16)
    misc = np.zeros((128, MISC_W), dtype=np.float32)
    for k in range(3):
        wT = conv_w[:, :, k].T.astype(np.float32)  # [i, o]
        misc[0:64, 128 * k : 128 * k + 64] = wT
        misc[64:128, 128 * k + 64 : 128 * k + 128] = -wT
    misc[:, IDENT : IDENT + 128] = np.eye(128, dtype=np.float32)
    misc = misc.astype(ml_dtypes.bfloat16)
    bias = np.concatenate([conv_b, -conv_b]).astype(np.float32).reshape(128, 1)
    misc[:, 384:386] = bias.view(np.uint32).view(ml_dtypes.bfloat16).reshape(128, 2)
    misc[:, XIN0 : XIN0 + 520] = x1[:, 0:520]
    return x1, misc


def kernel(x, conv_w, conv_b, _trace=False):
    x = np.asarray(x)
    conv_w = np.asarray(conv_w)
    conv_b = np.asarray(conv_b)
    x1, misc = _host_prep(x, conv_w, conv_b)

    in_maps = []
    for r in range(NCORES):
        mr = misc.copy()
        mr[:, X1Q:] = x1[:, 1 + r * ISL : 1 + (r + 1) * ISL]
        in_maps.append({"xin": x1, "misc": mr})

    nc = _get_nc()
    res = run_bass_kernel_spmd(nc, in_maps, list(range(NCORES)), trace=_trace)
    out = np.concatenate([res.results[r]["out"] for r in range(NCORES)], axis=1)
    out = np.asarray(out).astype(np.float32).reshape(B, C, 16, 16, 16)
    if _trace:
        return out, res
    return out


# revision 49
# speedup vs baseline: 1.2129x; 1.2129x over previous
"""DSPA (dual-softmax sparse attention) Trainium2 kernel.

Math (reference):
  x1 = x.reshape(2, 64, 4096)                       # [b, c, n]
  x2 = dilated_conv1d(x1, w, b) (k=3, dil=2, pad=1) # [b, c, m], m = n-2
  s[b] = x1[b]^T @ x2[b]                            # [n, m]
  x3 = softmax over b (2 elements)  ->  x3[0] = sigmoid(s0 - s1), x3[1] = 1 - x3[0]
  x4[b] = x2[b] @ x3[b]^T                           # [c, n]
  out = x1 + x4

Key identities used:
  p := sigmoid(d),  d := s0 - s1
  x4[0] = x2[0] @ p^T
  x4[1] = S1 - x2[1] @ p^T          with S1[c] = sum_j x2[1][c, j]
and in fp8 mode, with q := p - 0.5 = 0.5*tanh(d/2):
  x4[0] = x2[0] @ q^T + 0.5*S0
  x4[1] = 0.5*S1 - x2[1] @ q^T

Device strategy (8 cores, no collectives): shard the query dim n=4096 ->
512 columns per core. Each core computes the full conv x2 on-chip in a
batch-stacked, batch1-NEGATED bf16 layout x2bf [128, 4096] so one matmul
with K=128 yields d^T tiles directly.

Scheduling notes (from HW traces):
 - Every DMA piece lands ~2us after its (serialized, ~0.7us) issue, so
   the first misc piece carries conv weights AND a copy of xin[:, 0:520]
   -- everything conv(0) needs -- and later pieces are ordered by first
   use.
 - The PE p-state resets on even ~0.4us idle gaps (costing us of
   re-ramp), so garbage-transpose warmup runs until the first piece
   lands, and conv runs TWO chunks ahead of the consuming pipeline so no
   PE instruction ever waits on the conv-evacuation DVE chain.
 - Stage-3 lags the d-matmuls by 3 groups so PE never waits on the ACT
   sigmoid.
"""

import numpy as np
from collections import deque

import concourse.bacc as bacc
import concourse.mybir as mybir
import concourse.tile as tile
from concourse.bass_utils import run_bass_kernel_spmd

F32 = mybir.dt.float32
BF16 = mybir.dt.bfloat16
F8E4 = mybir.dt.float8e4

B, C, N, M = 2, 64, 4096, 4094
NCORES = 8
ISL = N // NCORES          # 512 query columns per core
NT = 32                    # j tiles of 128 (last has 2 zero cols)
NCH = 8                    # conv chunks of 512 (last = 510)
# misc layout: [w0 w1 w2 (384) | bias (2) | xin0copy (520) | ident (128)
#               | x1q (512)]
XIN0 = 386                 # xin[:, 0:520] copy offset
IDENT = 906
X1Q = 1034
MISC_W = X1Q + ISL         # 1546
NWARM = 26                 # PE p-state warmup transposes
FP8_S3 = True              # stage-3 matmuls in fp8e4 DoubleRow (K=256)

ADD = mybir.AluOpType.add
MULT = mybir.AluOpType.mult


def build_nc():
    nc = bacc.Bacc()

    xin = nc.dram_tensor("xin", [128, N + 2], BF16, kind="ExternalInput")
    misc = nc.dram_tensor("misc", [128, MISC_W], BF16, kind="ExternalInput")
    out = nc.dram_tensor("out", [128, ISL], BF16, kind="ExternalOutput")

    with tile.TileContext(nc) as tc:
        with (
            tc.tile_pool(name="persist", bufs=1) as persist,
            tc.tile_pool(name="ptiles", bufs=16) as ptiles,
            tc.tile_pool(name="psum_acc", bufs=1, space="PSUM") as psum_acc,
        ):
            x1pads = persist.tile([128, N + 2], BF16)
            msb = persist.tile([128, MISC_W], BF16)
            x2bf = persist.tile([128, NT * 128], BF16)
            x2T = persist.tile([128, NT * 128], F8E4 if FP8_S3 else BF16)
            wsrc = persist.tile([128, 128], BF16)
            scratch = persist.tile([128, 1], F32)
            s1parts = persist.tile([128, NCH + 1], F32)
            s1corr = persist.tile([128, 1], F32)
            s1half = persist.tile([128, 1], F32)
            osb = persist.tile([128, ISL], BF16)

            def wsb(k):
                return msb[:, 128 * k : 128 * (k + 1)]

            bsb = msb[:, 384:386].bitcast(F32)
            isb = msb[:, IDENT : IDENT + 128]
            x1q = msb[:, X1Q : X1Q + ISL]

            # Early memsets on the (otherwise idle) GPSIMD engine.
            nc.gpsimd.memset(wsrc[:, :], 0.0)
            nc.gpsimd.memset(scratch[:, :], 0.0)
            nc.gpsimd.memset(x2bf[:, M : NT * 128], 0.0)
            nc.gpsimd.memset(s1corr[:, :], 0.0)

            # Dummy activation: forces the ACT table DMA at kernel start.
            nc.scalar.activation(
                scratch[:, :], scratch[:, :],
                mybir.ActivationFunctionType.Tanh if FP8_S3
                else mybir.ActivationFunctionType.Sigmoid,
            )

            # Input DMAs ordered by first use. Piece 1 alone unblocks
            # conv(0); piece 2 (ident + x1q) unblocks d(0)/tr(0); piece 3
            # unblocks conv(1).
            nc.sync.dma_start(msb[:, 0:IDENT], misc[:, 0:IDENT])
            nc.sync.dma_start(msb[:, IDENT:], misc[:, IDENT:])
            nc.sync.dma_start(x1pads[:, 512:1028], xin[:, 512:1028])
            nc.sync.dma_start(x1pads[:, 1028:1720], xin[:, 1028:1720])
            nc.sync.dma_start(x1pads[:, 1720:2920], xin[:, 1720:2920])
            nc.sync.dma_start(x1pads[:, 2920:], xin[:, 2920:])

            # PE p-state warmup: garbage 128-row transposes keep the PE busy
            # (and ramping to 2.4GHz) until the first input piece lands.
            with tc.tile_pool(name="psum_warm", bufs=1, space="PSUM") as psum_warm:
                wps = psum_warm.tile([128, 128], BF16)
                for _ in range(NWARM):
                    nc.tensor.transpose(wps[:, :], wsrc[:, :], wsrc[:, :])

            acc = psum_acc.tile([128, ISL], F32)
            with (
                tc.tile_pool(name="psum_conv", bufs=2, space="PSUM") as psum_conv,
                tc.tile_pool(name="psum_tr", bufs=1, space="PSUM") as psum_tr,
                tc.tile_pool(name="psum_d", bufs=2, space="PSUM") as psum_d,
            ):
                pmap = {}
                s3q = deque()
                state = {"first": True}

                def emit_s3(g, last=False):
                    p = pmap.pop(g)
                    if FP8_S3:
                        lhs = x2T[:, 256 * g : 256 * (g + 1)].rearrange(
                            "p (t n) -> p t n", t=2
                        )
                        rhs = p[:, :].rearrange("p (t n) -> p t n", t=2)
                        nc.tensor.matmul(
                            acc[:, :], lhs, rhs,
                            start=state["first"], stop=last,
                            perf_mode=mybir.MatmulPerfMode.DoubleRow,
                        )
                        state["first"] = False
                    else:
                        for u in (0, 1):
                            t = 2 * g + u
                            nc.tensor.matmul(
                                acc[:, :],
                                x2T[:, 128 * t : 128 * (t + 1)],
                                p[:, 512 * u : 512 * u + 512],
                                start=state["first"], stop=(last and u == 1),
                            )
                            state["first"] = False

                def emit_conv(ch):
                    # Conv: x2bf[:, j] = sum_k wblkT[k].T @ x1pads[:, j + 2k]
                    # (block-diagonal weights, batch-1 block negated), plus
                    # its DVE evacuation (+bias, per-chunk row sums). Chunk
                    # 0 reads the xin copy embedded in misc piece 1, and its
                    # evacuation is split in halves so the first d-pair (and
                    # hence the first sigmoid) starts half an evac earlier.
                    j0 = ch * 512
                    w = min(512, M - j0)
                    pc = psum_conv.tile([128, 512], F32, name=f"pc{ch}", tag="pc")
                    for k in range(3):
                        if ch == 0:
                            rhs = msb[:, XIN0 + 2 * k : XIN0 + 2 * k + w]
                        else:
                            rhs = x1pads[:, j0 + 2 * k : j0 + 2 * k + w]
                        nc.tensor.matmul(
                            pc[:, 0:w], wsb(k), rhs,
                            start=(k == 0), stop=(k == 2),
                        )
                    halves = [(0, 256, 0), (256, 512, 1)] if ch == 0 else [(0, w, ch + 1)]
                    for a, b, col in halves:
                        nc.vector.tensor_scalar(
                            x2bf[:, j0 + a : j0 + b],
                            pc[:, a:b],
                            bsb,
                            0.0,
                            op0=ADD,
                            op1=ADD,
                            accum_out=s1parts[:, col : col + 1],
                        )

                # Conv runs ~2 chunks ahead of the consuming pipeline so the
                # transposes/d-matmuls never wait on the conv evacuation
                # chain, and the PE never idles waiting for ACT/DVE.
                # conv(1)/conv(2) go at iteration 0's tail (their DMA pieces
                # land after d(0)'s inputs; d(0) must not queue behind them).
                emit_conv(0)

                def emit_d(gg):
                    # d^T tiles + sigmoid/tanh. High priority: the sigmoid
                    # chain is the saturated resource, so the scheduler must
                    # prefer d-matmuls over transposes/conv when both are
                    # ready.
                    with tc.high_priority(offset=30):
                        dps = psum_d.tile([128, 1024], F32, name=f"d{gg}", tag="d")
                        for u in (0, 1):
                            t = 2 * gg + u
                            nc.tensor.matmul(
                                dps[:, 512 * u : 512 * u + 512],
                                x2bf[:, 128 * t : 128 * t + 128],
                                x1q,
                                start=True,
                                stop=True,
                            )
                        p = ptiles.tile(
                            [128, 1024], F8E4 if FP8_S3 else BF16,
                            name=f"p{gg}", tag="p",
                        )
                        if FP8_S3:
                            nc.scalar.activation(
                                p[:, :], dps[:, :],
                                mybir.ActivationFunctionType.Tanh, scale=0.5,
                            )
                        else:
                            nc.scalar.activation(
                                p[:, :], dps[:, :],
                                mybir.ActivationFunctionType.Sigmoid,
                            )
                        pmap[gg] = p
                        s3q.append(gg)

                for ch in range(NCH):
                    # Both d groups at the absolute iteration head: their
                    # sigmoids are the critical resource (ACT is saturated),
                    # and the d-pair must complete before ACT finishes the
                    # previous group's sigmoid.
                    emit_d(2 * ch)
                    emit_d(2 * ch + 1)
                    # Lagged stage-3: sigmoid of these groups completed at
                    # least one chunk ago.
                    while s3q and s3q[0] <= 2 * ch - 3:
                        emit_s3(s3q.popleft())
                    # x2^T tiles: 4 PE transposes into one PSUM tile, one
                    # DVE evacuation.
                    trt = psum_tr.tile([128, 1024], BF16, name=f"tr{ch}", tag="tr")
                    for tt in range(4):
                        t = 4 * ch + tt
                        nc.tensor.transpose(
                            trt[:, 128 * tt : 128 * (tt + 1)],
                            x2bf[:, 128 * t : 128 * (t + 1)],
                            isb,
                        )
                    if FP8_S3:
                        # x2T tiles scaled by 0.5 (folds q = 0.5*tanh(d/2)).
                        nc.vector.tensor_scalar(
                            x2T[:, 512 * ch : 512 * ch + 512], trt[:, 0:512],
                            0.5, 0.0, op0=MULT, op1=ADD,
                        )
                    else:
                        nc.vector.tensor_copy(
                            x2T[:, 512 * ch : 512 * ch + 512], trt[:, 0:512]
                        )
                    # All conv chunks front-loaded into iterations 0-2 (each
                    # gated only by its DMA piece landing): iterations 3-7
                    # are conv-free, so PE per iter drops below ACT's
                    # ~2.05us and the sigmoid chain runs bubble-free.
                    if ch == 0:
                        emit_conv(1)
                        emit_conv(2)
                    elif ch == 1:
                        emit_conv(3)
                        emit_conv(4)
                        emit_conv(5)
                    elif ch == 2:
                        emit_conv(6)
                        emit_conv(7)
                    if ch == NCH - 1:
                        # Corrections, ready well before the epilogue.
                        if FP8_S3:
                            # corr = [0.5*S0 ; 0.5*S1]; s1parts rows hold
                            # [S0_chunk ; -S1_chunk].
                            nc.vector.reduce_sum(
                                s1corr[0:64, 0:1], s1parts[0:64, :],
                                axis=mybir.AxisListType.X,
                            )
                            nc.vector.reduce_sum(
                                s1corr[64:128, 0:1], s1parts[64:128, :],
                                axis=mybir.AxisListType.X, negate=True,
                            )
                            nc.vector.tensor_scalar(
                                s1half[:, :], s1corr[:, :], 0.5, 0.0,
                                op0=MULT, op1=ADD,
                            )
                        else:
                            # corr = [0 ; S1] (s1corr top was memset to 0).
                            nc.vector.reduce_sum(
                                s1corr[64:128, 0:1], s1parts[64:128, :],
                                axis=mybir.AxisListType.X, negate=True,
                            )
                while s3q:
                    g = s3q.popleft()
                    emit_s3(g, last=(len(s3q) == 0))
                # Epilogue: out = (acc + corr) + x1, in two halves so the
                # first half's DMA issue overlaps the second half's DVE pass.
                corr = s1half if FP8_S3 else s1corr
                for a, b in ((0, 256), (256, ISL)):
                    nc.vector.scalar_tensor_tensor(
                        osb[:, a:b],
                        acc[:, a:b],
                        corr[:, 0:1],
                        x1q[:, a:b],
                        op0=ADD,
                        op1=ADD,
                    )
                    nc.sync.dma_start(out[:, a:b], osb[:, a:b])

    nc.finalize()
    return nc


_NC_CACHE = None


def _get_nc():
    global _NC_CACHE
    if _NC_CACHE is None:
        _NC_CACHE = build_nc()
    return _NC_CACHE


def _host_prep(x, conv_w, conv_b):
    import ml_dtypes

    x1 = np.zeros((B * C, N + 2), dtype=np.float32)
    x1[:, 1 : N + 1] = x.reshape(B * C, N)
    x1 = x1.astype(ml_dtypes.bfloat16)
    misc = np.zeros((128, MISC_W), dtype=np.float32)
    for k in range(3):
        wT = conv_w[:, :, k].T.astype(np.float32)  # [i, o]
        misc[0:64, 128 * k : 128 * k + 64] = wT
        misc[64:128, 128 * k + 64 : 128 * k + 128] = -wT
    misc[:, IDENT : IDENT + 128] = np.eye(128, dtype=np.float32)
    misc = misc.astype(ml_dtypes.bfloat16)
    bias = np.concatenate([conv_b, -conv_b]).astype(np.float32).reshape(128, 1)
    misc[:, 384:386] = bias.view(np.uint32).view(ml_dtypes.bfloat16).reshape(128, 2)
    misc[:, XIN0 : XIN0 + 520] = x1[:, 0:520]
    return x1, misc


def kernel(x, conv_w, conv_b, _trace=False):
    x = np.asarray(x)
    conv_w = np.asarray(conv_w)
    conv_b = np.asarray(conv_b)
    x1, misc = _host_prep(x, conv_w, conv_b)

    in_maps = []
    for r in range(NCORES):
        mr = misc.copy()
        mr[:, X1Q:] = x1[:, 1 + r * ISL : 1 + (r + 1) * ISL]
        in_maps.append({"xin": x1, "misc": mr})

    nc = _get_nc()
    res = run_bass_kernel_spmd(nc, in_maps, list(range(NCORES)), trace=_trace)
    out = np.concatenate([res.results[r]["out"] for r in range(NCORES)], axis=1)
    out = np.asarray(out).astype(np.float32).reshape(B, C, 16, 16, 16)
    if _trace:
        return out, res
    return out
